# revision 1
# baseline (speedup 1.0000x reference)
"""Sliding-window (W=128) multi-head attention block for Trainium2, 8 cores.

Reference computation (B=2, T=2048, E=1024, H=16, D=64, W=128):
    qkv = x @ w_qkv.T ; split q,k,v ; heads ; att = softmax(mask(q k^T / 8)) v
    out = att_concat @ w_out.T

Sharding: data-parallel over B (2) x tensor-parallel over head groups (4),
so each of the 8 cores handles (one batch, 4 heads).  The output projection
is computed per-core against the 256 w_out columns belonging to its heads,
giving a partial [T, E] output; the host sums the 4 partials per batch.

Per-core device layouts (all bf16, pre-arranged on host):
    xT   [E, T]    : x[b] transposed  (contraction dim E on partitions)
    wqk  [E, 512]  : [w_q_rows ; w_k_rows].T for the 4 heads
    wv   [E, 256]  : w_v_rows.T for the 4 heads
    wout [256, E]  : w_out[:, head_cols].T
    outp [T, E] f32: partial output

The sliding window of 128 means a 128-row query tile qi only sees key tiles
qi-1 and qi, with complementary triangular masks -> banded flash-style
attention with no online softmax needed.
"""

import numpy as np
import ml_dtypes

import concourse.bass as bass
import concourse.bacc as bacc
import concourse.mybir as mybir
import concourse.tile as tile
from concourse.bass_utils import run_bass_kernel_spmd
from concourse.masks import make_identity

B, T, E, H, W = 2, 2048, 1024, 16, 128
D = E // H            # 64
HPC = 4               # heads per core
N_CORES = 8
SCALE = 1.0 / float(np.sqrt(D))

BF16 = mybir.dt.bfloat16
F32 = mybir.dt.float32
NEG = -1.0e9

KO = E // 128         # 8 contraction chunks
NQT = T // 128        # 16 query tiles
NT512 = T // 512      # 4 tiles for the qk projection


def build_bass():
    # Bacc (not plain Bass): its finalize() runs the wait-redistribution
    # passes (event semaphores, matmul->ldweights wait moves) that the
    # 1-wait-per-instruction hardware encoding requires.
    nc = bacc.Bacc()
    xT = nc.declare_dram_parameter("xT", [E, T], BF16, isOutput=False)
    wqk = nc.declare_dram_parameter("wqk", [E, 2 * HPC * D], BF16, isOutput=False)
    wv = nc.declare_dram_parameter("wv", [E, HPC * D], BF16, isOutput=False)
    wout = nc.declare_dram_parameter("wout", [HPC * D, E], BF16, isOutput=False)
    outp = nc.declare_dram_parameter("outp", [T, E], F32, isOutput=True)

    with tile.TileContext(nc) as tc:
        with (
            tc.tile_pool(name="consts", bufs=1) as consts,
            tc.tile_pool(name="persist", bufs=1) as persist,
            tc.tile_pool(name="work", bufs=2) as work,
            tc.tile_pool(name="outw", bufs=3) as outw,
            tc.tile_pool(name="ps_mm", bufs=2, space="PSUM") as ps_mm,
            tc.tile_pool(name="ps_s", bufs=1, space="PSUM") as ps_s,
            tc.tile_pool(name="ps_pt", bufs=2, space="PSUM") as ps_pt,
            tc.tile_pool(name="ps_ot", bufs=1, space="PSUM") as ps_ot,
        ):
            # ---- constants ----
            ident = consts.tile([128, 128], BF16)
            make_identity(nc, ident)

            # Additive masks for a [128 q, 256 key] block, keys = [prev | cur].
            # main (qi>0):  prev half valid iff jl >= il+1, cur half causal.
            # first (qi=0): keys are [0:256]; valid iff j <= il (causal | none).
            mask_main = consts.tile([128, 256], F32)
            mask_first = consts.tile([128, 256], F32)
            # prev half: start from NEG, keep NEG where il - jl >= 0 (jl <= il)
            nc.gpsimd.memset(mask_main[:, 0:128], NEG)
            nc.gpsimd.affine_select(
                out=mask_main[:, 0:128], in_=mask_main[:, 0:128],
                compare_op=mybir.AluOpType.is_ge, fill=0.0,
                base=0, pattern=[[-1, 128]], channel_multiplier=1,
            )
            # cur half: start from 0, keep 0 where jl <= il, else NEG
            nc.gpsimd.memset(mask_main[:, 128:256], 0.0)
            nc.gpsimd.affine_select(
                out=mask_main[:, 128:256], in_=mask_main[:, 128:256],
                compare_op=mybir.AluOpType.is_ge, fill=NEG,
                base=0, pattern=[[-1, 128]], channel_multiplier=1,
            )
            nc.gpsimd.memset(mask_first[:, 0:128], 0.0)
            nc.gpsimd.affine_select(
                out=mask_first[:, 0:128], in_=mask_first[:, 0:128],
                compare_op=mybir.AluOpType.is_ge, fill=NEG,
                base=0, pattern=[[-1, 128]], channel_multiplier=1,
            )
            nc.gpsimd.memset(mask_first[:, 128:256], NEG)

            # ---- weight + x loads ----
            wqk_sb = persist.tile([128, KO, 2 * HPC * D], BF16)
            wv_sb = persist.tile([128, KO, HPC * D], BF16)
            wout_sb = persist.tile([128, 2, E], BF16)
            nc.sync.dma_start(
                out=wqk_sb, in_=wqk[:, :].rearrange("(ko p) m -> p ko m", p=128))
            nc.sync.dma_start(
                out=wv_sb, in_=wv[:, :].rearrange("(ko p) m -> p ko m", p=128))
            nc.sync.dma_start(
                out=wout_sb, in_=wout[:, :].rearrange("(c p) m -> p c m", p=128))

            xT_sb = persist.tile([128, KO, T], BF16)
            x_ap = xT[:, :].rearrange("(ko p) t -> p ko t", p=128)
            for ko in range(KO):
                nc.sync.dma_start(out=xT_sb[:, ko, :], in_=x_ap[:, ko, :])

            # persistent activations.  To keep every matmul operand at base
            # partition 0 (partition-offset PE operands are untested on HW),
            # k^T and v are stored zero-padded per head parity: head h
            # occupies partitions/cols (h%2)*64..+64, the rest is zero, and
            # K=128 contractions over the zeros are exact.
            qkT_sb = persist.tile([128, 2, T], BF16)   # q^T chunks: q01,q23
            kTz_sb = persist.tile([128, HPC, T], BF16)  # per-head padded k^T
            vz_sb = persist.tile([128, NQT, HPC, 128], BF16)  # padded v
            attT_sb = persist.tile([128, 2, T], BF16)  # O^T stacked [c, t]
            nc.vector.memset(kTz_sb, 0.0)
            nc.gpsimd.memset(vz_sb, 0.0)

            # ---- stage 1: qkv projections ----
            for ti in range(NT512):
                tsl = slice(ti * 512, (ti + 1) * 512)
                for mi in range(4):
                    ps = ps_mm.tile([128, 512], F32, tag="mm")
                    for ko in range(KO):
                        nc.tensor.matmul(
                            ps,
                            lhsT=wqk_sb[:, ko, mi * 128:(mi + 1) * 128],
                            rhs=xT_sb[:, ko, tsl],
                            start=(ko == 0), stop=(ko == KO - 1),
                        )
                    if mi < 2:
                        # q chunks: fold in 1/sqrt(D) while casting (ACT)
                        nc.scalar.activation(
                            out=qkT_sb[:, mi, tsl], in_=ps,
                            func=mybir.ActivationFunctionType.Copy, scale=SCALE)
                    else:
                        hp = (mi - 2) * 2   # heads hp, hp+1 in this chunk
                        nc.vector.tensor_copy(
                            out=kTz_sb[0:64, hp, tsl], in_=ps[0:64])
                        nc.vector.tensor_copy(
                            out=kTz_sb[64:128, hp + 1, tsl], in_=ps[64:128])
                for j in range(4):
                    t0 = ti * 512 + j * 128
                    ps = ps_mm.tile([128, 512], F32, tag="mm")
                    for ko in range(KO):
                        nc.tensor.matmul(
                            ps[:, 0:HPC * D],
                            lhsT=xT_sb[:, ko, t0:t0 + 128],
                            rhs=wv_sb[:, ko, :],
                            start=(ko == 0), stop=(ko == KO - 1),
                        )
                    for h in range(HPC):
                        nc.vector.tensor_copy(
                            out=vz_sb[:, ti * 4 + j, h,
                                      (h % 2) * 64:(h % 2) * 64 + 64],
                            in_=ps[:, h * D:(h + 1) * D])

            # ---- stage 2: banded attention ----
            for qi in range(NQT):
                kw = max(0, qi - 1) * 128      # first key of the 256-key window
                mask = mask_first if qi == 0 else mask_main
                qsl = slice(qi * 128, (qi + 1) * 128)

                s_ps = ps_s.tile([128, HPC, 256], F32, tag="S")
                for h in range(HPC):
                    nc.tensor.matmul(
                        s_ps[:, h, :],
                        lhsT=qkT_sb[:, h // 2, qsl],
                        rhs=kTz_sb[:, h, kw:kw + 256],
                        start=True, stop=True,
                    )

                sm = work.tile([128, HPC, 256], F32, tag="Sm")
                for h in range(HPC):
                    nc.vector.tensor_tensor(
                        sm[:, h, :], s_ps[:, h, :], mask, mybir.AluOpType.add)
                esb = work.tile([128, HPC, 256], F32, tag="E")
                lsb = work.tile([128, HPC], F32, tag="l")
                for h in range(HPC):
                    nc.scalar.activation(
                        out=esb[:, h, :], in_=sm[:, h, :],
                        func=mybir.ActivationFunctionType.Exp,
                        accum_out=lsb[:, h:h + 1],
                    )
                rl = work.tile([128, HPC], F32, tag="rl")
                nc.vector.reciprocal(out=rl, in_=lsb)
                psb = work.tile([128, HPC, 256], BF16, tag="P")
                for h in range(HPC):
                    nc.scalar.activation(
                        out=psb[:, h, :], in_=esb[:, h, :],
                        func=mybir.ActivationFunctionType.Copy,
                        scale=rl[:, h:h + 1],
                    )

                ot_ps = ps_ot.tile([128, 2, 128], F32, tag="OT")
                for h in range(HPC):
                    pt_ps = ps_pt.tile([128, 2, 128], BF16, tag="PT")
                    for ci in range(2):
                        nc.tensor.transpose(
                            pt_ps[:, ci, :],
                            psb[:, h, ci * 128:(ci + 1) * 128],
                            ident,
                        )
                    pt_sb = work.tile([128, 2, 128], BF16, tag="PTs")
                    nc.vector.tensor_copy(out=pt_sb, in_=pt_ps)
                    for ci in range(2):
                        nc.tensor.matmul(
                            ot_ps[:, h // 2, :],
                            lhsT=vz_sb[:, kw // 128 + ci, h, :],
                            rhs=pt_sb[:, ci, :],
                            start=(h % 2 == 0 and ci == 0),
                            stop=(h % 2 == 1 and ci == 1),
                        )
                nc.vector.tensor_copy(out=attT_sb[:, :, qsl], in_=ot_ps)

            # ---- stage 3: output projection (partial over this core's heads) --
            for ti in range(NQT):
                tsl = slice(ti * 128, (ti + 1) * 128)
                o_sb = outw.tile([128, E], F32, tag="osb")
                for nh in range(2):
                    po = ps_mm.tile([128, 512], F32, tag="mm")
                    for ko in range(2):
                        nc.tensor.matmul(
                            po,
                            lhsT=attT_sb[:, ko, tsl],
                            rhs=wout_sb[:, ko, nh * 512:(nh + 1) * 512],
                            start=(ko == 0), stop=(ko == 1),
                        )
                    if nh == 0:
                        nc.vector.tensor_copy(
                            out=o_sb[:, nh * 512:(nh + 1) * 512], in_=po)
                    else:
                        nc.scalar.copy(
                            out=o_sb[:, nh * 512:(nh + 1) * 512], in_=po)
                nc.sync.dma_start(out=outp[tsl, :], in_=o_sb)

    nc.finalize()
    return nc


_NC_CACHE = None


def _get_nc():
    global _NC_CACHE
    if _NC_CACHE is None:
        _NC_CACHE = build_bass()
    return _NC_CACHE


def make_in_maps(x, w_qkv, w_out):
    x = np.asarray(x, dtype=np.float32)
    w_qkv = np.asarray(w_qkv, dtype=np.float32)
    w_out = np.asarray(w_out, dtype=np.float32)
    bf = ml_dtypes.bfloat16
    in_maps = []
    for c in range(N_CORES):
        b = c // 4
        hs = (c % 4) * HPC
        rows = slice(hs * D, (hs + HPC) * D)
        wq = w_qkv[0 * E:, :][rows]             # [256, E]
        wk = w_qkv[1 * E:, :][rows]
        wvs = w_qkv[2 * E:, :][rows]
        in_maps.append({
            "xT": np.ascontiguousarray(x[b].T).astype(bf),
            "wqk": np.ascontiguousarray(
                np.concatenate([wq, wk], axis=0).T).astype(bf),
            "wv": np.ascontiguousarray(wvs.T).astype(bf),
            "wout": np.ascontiguousarray(w_out[:, rows].T).astype(bf),
        })
    return in_maps


def run(x, w_qkv, w_out, **spmd_kwargs):
    nc = _get_nc()
    in_maps = make_in_maps(x, w_qkv, w_out)
    res = run_bass_kernel_spmd(nc, in_maps, core_ids=list(range(N_CORES)),
                               **spmd_kwargs)
    outs = [r["outp"] for r in res.results]
    out = np.empty((B, T, E), dtype=np.float32)
    for b in range(B):
        acc = outs[4 * b].astype(np.float32)
        for c in range(4 * b + 1, 4 * b + 4):
            acc = acc + outs[c]
        out[b] = acc
    return out, res


def kernel(x, w_qkv, w_out):
    out, _ = run(x, w_qkv, w_out)
    return out

